# revision 22
# baseline (speedup 1.0000x reference)
"""Trainium2 Bass kernel for nn_DivEncLayer (grouped tiny-MLP + ELU + LayerNorm + proj).

Math (per batch row b, slice q of Q=128, V=8, H=32):
    h   = elu(x[b,q,:] @ W1[q] + b1[q]);  hn = LN(h)*gamma[q]+beta[q]
    out[b,q] = hn @ W2[q] + b2[q]

Folded form (LN algebra -> 3 segmented reductions, all done by PE matmuls):
    g2c = gamma*W2 - mean(gamma*W2); c2 = sum(beta*W2)+b2
    s = sum_h(he), w = sum_h(he*g2c), t = sum_h(he^2)
    out = c2 + w * sqrt(H) / sqrt(t - s^2/H + H*eps)

Device layout: features on partitions, batch on free dim (host pre-transposes
x, so zero on-chip transposes). Per 512-batch supertile:
  - mm1: 32 block-diagonal [K=128, M=128, N=512] float32r matmuls (full rate)
  - ACT: emu = Exp(h+b1) (one pass)
  - DVE: het = relu(h+b1) + min(emu-1, 0)  == elu  (one fused custom op)
         he2 = het*het (bf16 tensor_tensor, 2x mode)
  - stats: 3 bf16 matmuls per tile, col-tiled (tile_position), zero-padded
    M=32 stationaries accumulating DENSE [128q, 512b] stats banks
  - finishing on dense banks: 2 custom DVE ops + Square/Ln/Exp on ACT
"""

import os
import sys

for _p in ("/opt/trn_rl_repo",):
    if _p not in sys.path:
        sys.path.insert(0, _p)

import numpy as np

B, Q, V, H = 32768, 128, 8, 32
N_CORES = 8
BC = B // N_CORES          # 4096 batch rows per core
SB = 512                   # supertile batch columns
NST = BC // SB             # 8 supertiles per core
LN_EPS = 1e-5

_CACHE = {}
_OPS_REGISTERED = False
_last_in_maps = None


def _q_of_r():
    # stats-bank row r = 32*t + 4*g + j  <->  q = 16*g + 4*t + j
    r = np.arange(128)
    t, g, j = r // 32, (r % 32) // 4, r % 4
    return (16 * g + 4 * t + j).astype(np.int64)


def _register_custom_ops():
    """Append our fused DVE ops to the dve_ops registry (self-pinned shas)."""
    global _OPS_REGISTERED
    import concourse.dve_ops as dve_ops
    from concourse.dve_ops import DveOp
    from concourse.dve_spec import C0, C1, Spec, Src0, Src1, Zero, lower, minn, relu
    from concourse.dve_uop import DveOpSpec

    if _OPS_REGISTERED:
        return {op.name: op for op in dve_ops.OPS}

    def _pin(name, spec, ref):
        spec = Spec(body=spec, reference=ref)
        shas = {}
        for ver in ("v3", "v4"):
            row = dve_ops._CUSTOM_DVE_ROW_BASE + len(dve_ops.OPS)
            tmp = DveOpSpec(name=name, opcode=row, uops=lower(spec, ver=ver),
                            rd1_en=True)
            shas[ver] = tmp.sha(ver)
        op = DveOp(name, spec, subdim=False, uops_sha=shas)
        dve_ops.OPS.append(op)
        dve_ops.CUSTOM_DVE_SPECS[name] = spec
        dve_ops._SUB_OPCODE_FOR_NAME[name] = dve_ops._CUSTOM_DVE_ROW_BASE + len(dve_ops.OPS) - 1
        return op

    # het = relu(h + b1) + min(emu - 1, 0)   (exact ELU given emu = exp(h+b1))
    _pin(
        "ELU_FUSE_ANT",
        relu(Src0 + C0) + minn(Src1 - C1, Zero),
        lambda in0, in1, s0, s1, imm2: np.maximum(in0.astype(np.float32) + s0, 0.0)
        + np.minimum(in1.astype(np.float32) - s1, 0.0),
    )
    # D = t - z*c0 + c1   (z = s^2 precomputed on ACT)
    _pin(
        "VAR_PREP_ANT",
        (Src0 - Src1 * C0) + C1,
        lambda in0, in1, s0, s1, imm2: (in0.astype(np.float32) - in1 * s0) + s1,
    )
    # out = rstd * w + c2
    _pin(
        "MUL_ADD_ANT",
        Src0 * Src1 + C0,
        lambda in0, in1, s0, s1, imm2: in0.astype(np.float32) * in1 + s0,
    )
    _OPS_REGISTERED = True
    return {op.name: op for op in dve_ops.OPS}

def _build_program(tile_dt_name: str, ablate: str = "", reps: int = 1):
    ab = set(ablate.split(",")) if ablate else set()
    import concourse.bacc as bacc
    import concourse.tile as tile
    from concourse import mybir

    ops = _register_custom_ops()

    f32 = mybir.dt.float32
    f32r = mybir.dt.float32r
    bf16 = mybir.dt.bfloat16
    tile_dt = getattr(mybir.dt, tile_dt_name)
    AF = mybir.ActivationFunctionType
    ALU = mybir.AluOpType

    nc = bacc.Bacc(
        "TRN2",
        target_bir_lowering=False,
        debug=False,
        enable_asserts=False,
        num_devices=N_CORES,
    )

    xT = nc.dram_tensor("xT", [Q * V, BC], f32r, kind="ExternalInput").ap()
    w1p = nc.dram_tensor("w1p", [128, 32 * 128], f32r, kind="ExternalInput").ap()
    sp = nc.dram_tensor("sp", [128, 32 * 128], f32r, kind="ExternalInput").ap()
    wp = nc.dram_tensor("wp", [128, 32 * 128], f32r, kind="ExternalInput").ap()
    b1p = nc.dram_tensor("b1p", [128, 32], f32, kind="ExternalInput").ap()
    c2p = nc.dram_tensor("c2p", [128, 1], f32, kind="ExternalInput").ap()
    outT = nc.dram_tensor("outT", [128, BC], f32, kind="ExternalOutput").ap()

    with tile.TileContext(nc) as tc:
        with (
            tc.tile_pool(name="wts", bufs=1) as wts,
            tc.tile_pool(name="xt", bufs=12) as xtp,
            tc.tile_pool(name="elu", bufs=4) as elu,
            tc.tile_pool(name="fin", bufs=2) as fin,
            tc.tile_pool(name="hep", bufs=2, space="PSUM") as hep,
            tc.tile_pool(name="stp", bufs=2, space="PSUM") as stp,
        ):
            w1s = wts.tile([128, 32 * 128], f32r)
            nc.sync.dma_start(out=w1s, in_=w1p)
            sps = wts.tile([128, 32 * 128], f32r)
            nc.sync.dma_start(out=sps, in_=sp)
            wps = wts.tile([128, 32 * 128], f32r)
            nc.sync.dma_start(out=wps, in_=wp)
            b1s = wts.tile([128, 32], f32)
            nc.sync.dma_start(out=b1s, in_=b1p)
            c2s = wts.tile([128, 1], f32)
            nc.sync.dma_start(out=c2s, in_=c2p)
            zero_c = wts.tile([128, 1], f32)
            nc.vector.memset(zero_c, 0.0)
            lnh_c = wts.tile([128, 1], f32)
            nc.vector.memset(lnh_c, float(0.5 * np.log(H)))

            import contextlib

            loop_cm = tc.For_i(0, reps, 1) if reps > 1 else contextlib.nullcontext()
            with loop_cm:
              for st in range(NST):
                xts = []
                for g in range(8):
                    xt_t = xtp.tile([128, SB], f32r, tag="xt")
                    nc.sync.dma_start(
                        out=xt_t, in_=xT[128 * g : 128 * g + 128, SB * st : SB * st + SB]
                    )
                    xts.append(xt_t)

                bankS = stp.tile([128, SB], f32, tag="bankS")
                bankW = stp.tile([128, SB], f32, tag="bankW")
                bankT = stp.tile([128, SB], f32, tag="bankT")

                for i in range(32):
                    g, t = i // 4, i % 4
                    he = hep.tile([128, SB], f32, tag="he")
                    nc.tensor.matmul(
                        he,
                        lhsT=w1s[:, 128 * i : 128 * i + 128],
                        rhs=xts[g],
                        start=True,
                        stop=True,
                    )
                    bias = b1s[:, i : i + 1]
                    emu = elu.tile([128, SB], f32, tag="emu")
                    nc.scalar.activation(emu, he, AF.Exp, bias=bias, scale=1.0)
                    het = elu.tile([128, SB], f32r, tag="het")
                    nc.vector._custom_dve(
                        ops["ELU_FUSE_ANT"], out=het, in0=he, in1=emu,
                        s0=bias, s1=1.0,
                    )
                    he2 = elu.tile([128, SB], f32r, tag="he2")
                    if i % 2 == 0:
                        nc.vector.tensor_mul(he2, het, het)
                    else:
                        nc.scalar.activation(he2, het, AF.Square, bias=zero_c[:, 0:1])

                    su = sps[:, 128 * i : 128 * i + 128]
                    wu = wps[:, 128 * i : 128 * i + 128]
                    first = i == 0
                    last = i == 31
                    if "stats" not in ab:
                        nc.tensor.matmul(bankS, lhsT=su, rhs=het, start=first, stop=last)
                        nc.tensor.matmul(bankW, lhsT=wu, rhs=het, start=first, stop=last)
                        nc.tensor.matmul(bankT, lhsT=su, rhs=he2, start=first, stop=last)
                    elif i == 0:
                        nc.tensor.matmul(bankS, lhsT=su, rhs=het, start=True, stop=True)
                        nc.tensor.matmul(bankW, lhsT=wu, rhs=het, start=True, stop=True)
                        nc.tensor.matmul(bankT, lhsT=su, rhs=he2, start=True, stop=True)

                # finishing: out = c2 + w * exp(0.5*ln(32) - 0.5*ln(D)),
                # D = t - s^2/32 + 32*eps
                z = fin.tile([128, SB], f32, tag="z")
                nc.scalar.activation(z, bankS, AF.Square, bias=zero_c[:, 0:1])
                D = fin.tile([128, SB], f32, tag="D")
                nc.vector._custom_dve(
                    ops["VAR_PREP_ANT"], out=D, in0=bankT, in1=z,
                    s0=1.0 / H, s1=float(H * LN_EPS),
                )
                L = fin.tile([128, SB], f32, tag="L")
                nc.scalar.activation(L, D, AF.Ln, bias=zero_c[:, 0:1])
                rstd = fin.tile([128, SB], f32, tag="rstd")
                nc.scalar.activation(rstd, L, AF.Exp, bias=lnh_c[:, 0:1], scale=-0.5)
                of = fin.tile([128, SB], f32, tag="of")
                nc.vector._custom_dve(
                    ops["MUL_ADD_ANT"], out=of, in0=rstd, in1=bankW,
                    s0=c2s[:, 0:1], s1=0.0,
                )
                nc.sync.dma_start(out=outT[:, SB * st : SB * st + SB], in_=of)

    nc.compile()
    return nc


def _host_pack(W1, b1, gamma, beta, W2, b2):
    import ml_dtypes

    g2 = (gamma * W2[:, :, 0]).astype(np.float64)
    g2c = (g2 - g2.sum(-1, keepdims=True) / H).astype(np.float32)
    c2 = ((beta * W2[:, :, 0]).sum(-1) + b2[:, 0]).astype(np.float32)

    w1p = np.zeros((128, 32 * 128), np.float32)
    sp = np.zeros((128, 32 * 128), np.float32)
    wp = np.zeros((128, 32 * 128), np.float32)
    b1p = np.zeros((128, 32), np.float32)
    for g in range(8):
        for t in range(4):
            i = 4 * g + t
            for j in range(4):
                q = 16 * g + 4 * t + j
                w1p[
                    32 * t + 8 * j : 32 * t + 8 * j + 8,
                    128 * i + 32 * j : 128 * i + 32 * j + 32,
                ] = W1[q]
                # bank row r = 32*t + 4*g + j; lhsT col m writes bank row m
                sp[32 * j : 32 * j + 32, 128 * i + 32 * t + 4 * g + j] = 1.0
                wp[32 * j : 32 * j + 32, 128 * i + 32 * t + 4 * g + j] = g2c[q]
                b1p[32 * j : 32 * j + 32, i] = b1[q]
    c2p = c2[_q_of_r()].reshape(128, 1).astype(np.float32)
    return (w1p, sp, wp, b1p, c2p)


def kernel(x, W1, b1, gamma, beta, W2, b2):
    from concourse import bass_utils

    tile_dt_name = os.environ.get("KERNEL_TILE_DT", "bfloat16")
    key = tile_dt_name
    if key not in _CACHE:
        _CACHE[key] = _build_program(tile_dt_name)
    nc = _CACHE[key]

    x = np.asarray(x, np.float32)
    w1p, sp, wp, b1p, c2p = _host_pack(
        np.asarray(W1, np.float32),
        np.asarray(b1, np.float32),
        np.asarray(gamma, np.float32),
        np.asarray(beta, np.float32),
        np.asarray(W2, np.float32),
        np.asarray(b2, np.float32),
    )

    in_maps = []
    for c in range(N_CORES):
        xc = x[BC * c : BC * (c + 1), :]          # [4096, 1024]
        in_maps.append(
            {
                "xT": np.ascontiguousarray(xc.T),  # [1024, 4096]
                "w1p": w1p,
                "sp": sp,
                "wp": wp,
                "b1p": b1p,
                "c2p": c2p,
            }
        )

    global _last_in_maps
    _last_in_maps = in_maps

    res = bass_utils.run_bass_kernel_spmd(
        nc, in_maps, core_ids=list(range(N_CORES))
    )

    qr = _q_of_r()
    out = np.empty((B, Q), np.float32)
    for c in range(N_CORES):
        blk = np.empty((BC, Q), np.float32)
        blk[:, qr] = res.results[c]["outT"].T
        out[BC * c : BC * (c + 1), :] = blk
    return out


# revision 23
# speedup vs baseline: 3.2767x; 3.2767x over previous
"""Trainium2 Bass kernel for nn_DivEncLayer (grouped tiny-MLP + ELU + LayerNorm + proj).

Math (per batch row b, slice q of Q=128, V=8, H=32):
    h   = elu(x[b,q,:] @ W1[q] + b1[q]);  hn = LN(h)*gamma[q]+beta[q]
    out[b,q] = hn @ W2[q] + b2[q]

Folded form (LN algebra -> 3 segmented reductions, all done by PE matmuls):
    g2c = gamma*W2 - mean(gamma*W2); c2 = sum(beta*W2)+b2
    s = sum_h(he), w = sum_h(he*g2c), t = sum_h(he^2)
    out = c2 + w * sqrt(H) / sqrt(t - s^2/H + H*eps)

Device layout: features on partitions, batch on free dim (host pre-transposes
x, so zero on-chip transposes). Per 512-batch supertile:
  - mm1: 32 block-diagonal [K=128, M=128, N=512] float32r matmuls (full rate)
  - ACT: emu = Exp(h+b1) (one pass)
  - DVE: het = relu(h+b1) + min(emu-1, 0)  == elu  (one fused custom op)
         he2 = het*het (bf16 tensor_tensor, 2x mode)
  - stats: 3 bf16 matmuls per tile, col-tiled (tile_position), zero-padded
    M=32 stationaries accumulating DENSE [128q, 512b] stats banks
  - finishing on dense banks: 2 custom DVE ops + Square/Ln/Exp on ACT
"""

import os
import sys

for _p in ("/opt/trn_rl_repo",):
    if _p not in sys.path:
        sys.path.insert(0, _p)

import numpy as np

B, Q, V, H = 32768, 128, 8, 32
N_CORES = 8
BC = B // N_CORES          # 4096 batch rows per core
SB = 512                   # supertile batch columns
NST = BC // SB             # 8 supertiles per core
LN_EPS = 1e-5

_CACHE = {}
_OPS_REGISTERED = False
_last_in_maps = None


def _q_of_r():
    # stats-bank row r = 32*t + 4*g + j  <->  q = 16*g + 4*t + j
    r = np.arange(128)
    t, g, j = r // 32, (r % 32) // 4, r % 4
    return (16 * g + 4 * t + j).astype(np.int64)


def _register_custom_ops():
    """Append our fused DVE ops to the dve_ops registry (self-pinned shas)."""
    global _OPS_REGISTERED
    import concourse.dve_ops as dve_ops
    from concourse.dve_ops import DveOp
    from concourse.dve_spec import C0, C1, Spec, Src0, Src1, Zero, lower, minn, relu
    from concourse.dve_uop import DveOpSpec

    if _OPS_REGISTERED:
        return {op.name: op for op in dve_ops.OPS}

    def _pin(name, spec, ref):
        spec = Spec(body=spec, reference=ref)
        shas = {}
        for ver in ("v3", "v4"):
            row = dve_ops._CUSTOM_DVE_ROW_BASE + len(dve_ops.OPS)
            tmp = DveOpSpec(name=name, opcode=row, uops=lower(spec, ver=ver),
                            rd1_en=True)
            shas[ver] = tmp.sha(ver)
        op = DveOp(name, spec, subdim=False, uops_sha=shas)
        dve_ops.OPS.append(op)
        dve_ops.CUSTOM_DVE_SPECS[name] = spec
        dve_ops._SUB_OPCODE_FOR_NAME[name] = dve_ops._CUSTOM_DVE_ROW_BASE + len(dve_ops.OPS) - 1
        return op

    # het = relu(h + b1) + min(emu - 1, 0)   (exact ELU given emu = exp(h+b1))
    _pin(
        "ELU_FUSE_ANT",
        relu(Src0 + C0) + minn(Src1 - C1, Zero),
        lambda in0, in1, s0, s1, imm2: np.maximum(in0.astype(np.float32) + s0, 0.0)
        + np.minimum(in1.astype(np.float32) - s1, 0.0),
    )
    # D = t - z*c0 + c1   (z = s^2 precomputed on ACT)
    _pin(
        "VAR_PREP_ANT",
        (Src0 - Src1 * C0) + C1,
        lambda in0, in1, s0, s1, imm2: (in0.astype(np.float32) - in1 * s0) + s1,
    )
    # out = rstd * w + c2
    _pin(
        "MUL_ADD_ANT",
        Src0 * Src1 + C0,
        lambda in0, in1, s0, s1, imm2: in0.astype(np.float32) * in1 + s0,
    )
    _OPS_REGISTERED = True
    return {op.name: op for op in dve_ops.OPS}

def _build_program(tile_dt_name: str, ablate: str = "", reps: int = 1):
    ab = set(ablate.split(",")) if ablate else set()
    import concourse.bacc as bacc
    import concourse.tile as tile
    from concourse import mybir

    ops = _register_custom_ops()

    f32 = mybir.dt.float32
    f32r = mybir.dt.float32r
    bf16 = mybir.dt.bfloat16
    tile_dt = getattr(mybir.dt, tile_dt_name)
    AF = mybir.ActivationFunctionType
    ALU = mybir.AluOpType

    nc = bacc.Bacc(
        "TRN2",
        target_bir_lowering=False,
        debug=False,
        enable_asserts=False,
        num_devices=N_CORES,
    )

    xT = nc.dram_tensor("xT", [Q * V, BC], f32r, kind="ExternalInput").ap()
    w1p = nc.dram_tensor("w1p", [128, 32 * 128], f32r, kind="ExternalInput").ap()
    sp = nc.dram_tensor("sp", [128, 32 * 128], f32r, kind="ExternalInput").ap()
    wp = nc.dram_tensor("wp", [128, 32 * 128], f32r, kind="ExternalInput").ap()
    b1p = nc.dram_tensor("b1p", [128, 32], f32, kind="ExternalInput").ap()
    c2p = nc.dram_tensor("c2p", [128, 1], f32, kind="ExternalInput").ap()
    outT = nc.dram_tensor("outT", [128, BC], f32, kind="ExternalOutput").ap()

    with tile.TileContext(nc) as tc:
        with (
            tc.tile_pool(name="wts", bufs=1) as wts,
            tc.tile_pool(name="xt", bufs=12) as xtp,
            tc.tile_pool(name="elu", bufs=4) as elu,
            tc.tile_pool(name="fin", bufs=2) as fin,
            tc.tile_pool(name="hep", bufs=2, space="PSUM") as hep,
            tc.tile_pool(name="stp", bufs=2, space="PSUM") as stp,
        ):
            w1s = wts.tile([128, 32 * 128], f32r)
            nc.sync.dma_start(out=w1s, in_=w1p)
            sps = wts.tile([128, 32 * 128], f32r)
            nc.sync.dma_start(out=sps, in_=sp)
            wps = wts.tile([128, 32 * 128], f32r)
            nc.sync.dma_start(out=wps, in_=wp)
            b1s = wts.tile([128, 32], f32)
            nc.sync.dma_start(out=b1s, in_=b1p)
            c2s = wts.tile([128, 1], f32)
            nc.sync.dma_start(out=c2s, in_=c2p)
            zero_c = wts.tile([128, 1], f32)
            nc.vector.memset(zero_c, 0.0)
            lnh_c = wts.tile([128, 1], f32)
            nc.vector.memset(lnh_c, float(0.5 * np.log(H)))

            import contextlib

            loop_cm = tc.For_i(0, reps, 1) if reps > 1 else contextlib.nullcontext()
            with loop_cm:
              for st in range(NST):
                xts = []
                for g in range(8):
                    xt_t = xtp.tile([128, SB], f32r, tag="xt")
                    nc.sync.dma_start(
                        out=xt_t, in_=xT[128 * g : 128 * g + 128, SB * st : SB * st + SB]
                    )
                    xts.append(xt_t)

                bankS = stp.tile([128, SB], f32, tag="bankS")
                bankW = stp.tile([128, SB], f32, tag="bankW")
                bankT = stp.tile([128, SB], f32, tag="bankT")

                for i in range(32):
                    g, t = i // 4, i % 4
                    he = hep.tile([128, SB], f32, tag="he")
                    nc.tensor.matmul(
                        he,
                        lhsT=w1s[:, 128 * i : 128 * i + 128],
                        rhs=xts[g],
                        start=True,
                        stop=True,
                    )
                    bias = b1s[:, i : i + 1]
                    emu = elu.tile([128, SB], f32, tag="emu")
                    nc.scalar.activation(emu, he, AF.Exp, bias=bias, scale=1.0)
                    het = elu.tile([128, SB], f32r, tag="het")
                    nc.vector._custom_dve(
                        ops["ELU_FUSE_ANT"], out=het, in0=he, in1=emu,
                        s0=bias, s1=1.0,
                    )
                    he2 = elu.tile([128, SB], f32r, tag="he2")
                    if "acthe2" in ab and i % 2 == 1:
                        nc.scalar.activation(he2, het, AF.Square, bias=zero_c[:, 0:1])
                    else:
                        nc.vector.tensor_mul(he2, het, het)

                    su = sps[:, 128 * i : 128 * i + 128]
                    wu = wps[:, 128 * i : 128 * i + 128]
                    first = i == 0
                    last = i == 31
                    if "stats" not in ab:
                        nc.tensor.matmul(bankS, lhsT=su, rhs=het, start=first, stop=last)
                        nc.tensor.matmul(bankW, lhsT=wu, rhs=het, start=first, stop=last)
                        nc.tensor.matmul(bankT, lhsT=su, rhs=he2, start=first, stop=last)
                    elif i == 0:
                        nc.tensor.matmul(bankS, lhsT=su, rhs=het, start=True, stop=True)
                        nc.tensor.matmul(bankW, lhsT=wu, rhs=het, start=True, stop=True)
                        nc.tensor.matmul(bankT, lhsT=su, rhs=he2, start=True, stop=True)

                # finishing: out = c2 + w * exp(0.5*ln(32) - 0.5*ln(D)),
                # D = t - s^2/32 + 32*eps
                z = fin.tile([128, SB], f32, tag="z")
                nc.scalar.activation(z, bankS, AF.Square, bias=zero_c[:, 0:1])
                D = fin.tile([128, SB], f32, tag="D")
                nc.vector._custom_dve(
                    ops["VAR_PREP_ANT"], out=D, in0=bankT, in1=z,
                    s0=1.0 / H, s1=float(H * LN_EPS),
                )
                L = fin.tile([128, SB], f32, tag="L")
                nc.scalar.activation(L, D, AF.Ln, bias=zero_c[:, 0:1])
                rstd = fin.tile([128, SB], f32, tag="rstd")
                nc.scalar.activation(rstd, L, AF.Exp, bias=lnh_c[:, 0:1], scale=-0.5)
                of = fin.tile([128, SB], f32, tag="of")
                nc.vector._custom_dve(
                    ops["MUL_ADD_ANT"], out=of, in0=rstd, in1=bankW,
                    s0=c2s[:, 0:1], s1=0.0,
                )
                nc.sync.dma_start(out=outT[:, SB * st : SB * st + SB], in_=of)

    nc.compile()
    return nc


def _host_pack(W1, b1, gamma, beta, W2, b2):
    import ml_dtypes

    g2 = (gamma * W2[:, :, 0]).astype(np.float64)
    g2c = (g2 - g2.sum(-1, keepdims=True) / H).astype(np.float32)
    c2 = ((beta * W2[:, :, 0]).sum(-1) + b2[:, 0]).astype(np.float32)

    w1p = np.zeros((128, 32 * 128), np.float32)
    sp = np.zeros((128, 32 * 128), np.float32)
    wp = np.zeros((128, 32 * 128), np.float32)
    b1p = np.zeros((128, 32), np.float32)
    for g in range(8):
        for t in range(4):
            i = 4 * g + t
            for j in range(4):
                q = 16 * g + 4 * t + j
                w1p[
                    32 * t + 8 * j : 32 * t + 8 * j + 8,
                    128 * i + 32 * j : 128 * i + 32 * j + 32,
                ] = W1[q]
                # bank row r = 32*t + 4*g + j; lhsT col m writes bank row m
                sp[32 * j : 32 * j + 32, 128 * i + 32 * t + 4 * g + j] = 1.0
                wp[32 * j : 32 * j + 32, 128 * i + 32 * t + 4 * g + j] = g2c[q]
                b1p[32 * j : 32 * j + 32, i] = b1[q]
    c2p = c2[_q_of_r()].reshape(128, 1).astype(np.float32)
    return (w1p, sp, wp, b1p, c2p)


def kernel(x, W1, b1, gamma, beta, W2, b2):
    from concourse import bass_utils

    tile_dt_name = os.environ.get("KERNEL_TILE_DT", "bfloat16")
    key = tile_dt_name
    if key not in _CACHE:
        _CACHE[key] = _build_program(tile_dt_name)
    nc = _CACHE[key]

    x = np.asarray(x, np.float32)
    w1p, sp, wp, b1p, c2p = _host_pack(
        np.asarray(W1, np.float32),
        np.asarray(b1, np.float32),
        np.asarray(gamma, np.float32),
        np.asarray(beta, np.float32),
        np.asarray(W2, np.float32),
        np.asarray(b2, np.float32),
    )

    in_maps = []
    for c in range(N_CORES):
        xc = x[BC * c : BC * (c + 1), :]          # [4096, 1024]
        in_maps.append(
            {
                "xT": np.ascontiguousarray(xc.T),  # [1024, 4096]
                "w1p": w1p,
                "sp": sp,
                "wp": wp,
                "b1p": b1p,
                "c2p": c2p,
            }
        )

    global _last_in_maps
    _last_in_maps = in_maps

    res = bass_utils.run_bass_kernel_spmd(
        nc, in_maps, core_ids=list(range(N_CORES))
    )

    qr = _q_of_r()
    out = np.empty((B, Q), np.float32)
    for c in range(N_CORES):
        blk = np.empty((BC, Q), np.float32)
        blk[:, qr] = res.results[c]["outT"].T
        out[BC * c : BC * (c + 1), :] = blk
    return out
